# revision 17
# baseline (speedup 1.0000x reference)
"""Trainium2 Bass kernel for nn_CausalRoPEAttention (windowed causal attention
with meta tokens, RoPE, GQA) on 8 NeuronCores.

Sharding: core c = b*4 + g  (b in {0,1} batch, g in {0..3} kv-head group).
Each core computes, for its batch b and its 4 query heads (= one GQA group):
  Q/K/V projections over full E, RoPE, windowed attention, and a partial
  output projection against its 512-row slice of Wo.  The host sums the 4
  partials per batch and adds bo.

v3 layout:
 - x arrives host-pre-transposed AND pre-bf16 as xT (E, S); Q and K are
   computed directly in [d, tok] orientation (weights stationary), so no
   on-device transposes of x / q / k at all.
 - RoPE runs in [d, tok] orientation: Q/K weight columns are host-permuted so
   each rotate-half partner sits 16 partitions away inside the same
   32-partition quadrant, making the rotation one DVE stream_shuffle; the
   cos / (sign-folded) sin tables are host-precomputed in the same layout.
 - V keeps natural [tok, d] orientation (x-tile stationary, bf16), and every
   V chunk carries an appended ones-column, so the attention AV matmul
   (query-major: p^T stationary, [V|1] moving) accumulates the softmax
   denominator for free in output column 128.  Normalization is then a
   per-partition reciprocal + scalar multiply; the row-sum ones-matmuls and
   the 1/s outer-product broadcast of v2 are gone.
 - Elementwise work is spread across Pool (gpsimd) / DVE / Act explicitly.
   1/sqrt(hd) is folded into Wq host-side.
"""

import numpy as np

import concourse.bass as bass
import concourse.mybir as mybir
import concourse.tile as tile
from concourse.masks import make_identity
from concourse.vector_clock import ScopedClock

# ---------------------------------------------------------------- constants
S = 4096
E = 2048
HD = 128
N_HEADS = 16
N_KV = 4
NQH = 4          # q heads per core
BLK = 512        # window size == query block
NBLK = S // BLK
META = 16
ROPE_BASE = 10000.0
F32 = mybir.dt.float32
BF = mybir.dt.bfloat16

VST = 132        # v_roll per-chunk column stride (128 v + 1 ones + 3 pad)
SHUF_ROT16 = [(i + 16) % 32 for i in range(32)]   # swap 16-halves per quadrant


class SafeTileContext(tile.TileContext):
    """TileContext whose final drain splits sem waits across nop instructions:
    this walrus caps non-EventSemaphore instructions at 1 sync wait."""

    def _drain_and_barrier(self, tick_clock, wait_clock):
        nc = self.nc
        dummy = nc.sync.nop(nofuse=True)
        wait_clock.add_sem_waits(dummy.ins, ScopedClock({None: tick_clock.global_clock}))
        waits = list(dummy.ins.sync_info.on_wait)
        if len(waits) > 1:
            dummy.ins.sync_info = mybir.SyncInfo(on_wait=waits[:1], on_update=[])
            for w in waits[1:]:
                nop = nc.sync.nop(nofuse=True)
                nop.ins.sync_info = mybir.SyncInfo(on_wait=[w], on_update=[])
        nc.sync.drain()
        nc.all_engine_barrier()
        popped = nc._tile_sem_poison_stack.pop()
        assert popped is self._sem_poison
        nc.clear_and_free_semaphores(list(self.sems.allocated().values()))
        nc.all_engine_barrier()
        self._split_excess_waits()

    def _split_excess_waits(self):
        """walrus (this version) allows 1 sync wait per instruction (2 on
        EventSemaphore); hoist extras onto preceding same-engine nops."""
        nc = self.nc
        for fn in nc.m.functions:
            for bb in fn.blocks:
                out = []
                changed = False
                for inst in bb.instructions:
                    si = inst.sync_info
                    waits = list(si.on_wait) if si is not None else []
                    cap = 2 if isinstance(inst, mybir.InstEventSemaphore) else 1
                    if len(waits) > cap:
                        changed = True
                        for w in waits[:-cap]:
                            nop = mybir.InstNoOp(
                                name=nc.get_next_instruction_name(), ins=[], outs=[])
                            nop.engine = inst.engine
                            nop.sync_info = mybir.SyncInfo(on_wait=[w], on_update=[])
                            nc.register_instruction(nop, overwrite=True)
                            out.append(nop)
                        inst.sync_info = mybir.SyncInfo(
                            on_wait=waits[-cap:], on_update=list(si.on_update))
                    out.append(inst)
                if changed:
                    bb.instructions = out


# ---------------------------------------------------------------- device IR
def build_nc(nblk=NBLK, passes=1):
    nc = bass.Bass()
    s = nblk * BLK

    xT_d = nc.dram_tensor("xt", (E, s), BF, kind="ExternalInput")
    wq_d = nc.dram_tensor("wq", (E, NQH * HD), BF, kind="ExternalInput")
    wkv_d = nc.dram_tensor("wkv", (E, 2 * HD), BF, kind="ExternalInput")
    wo_d = nc.dram_tensor("wo", (NQH * HD, E), BF, kind="ExternalInput")
    bq_d = nc.dram_tensor("bqc", (128, NQH), F32, kind="ExternalInput")
    bkv_d = nc.dram_tensor("bkvc", (128, 2), F32, kind="ExternalInput")
    bvb_d = nc.dram_tensor("bvb", (128, 128), F32, kind="ExternalInput")
    cos_d = nc.dram_tensor("cosd", (128, s), BF, kind="ExternalInput")
    sin_d = nc.dram_tensor("sind", (128, s), BF, kind="ExternalInput")
    y_d = nc.dram_tensor("y", (s, E), F32, kind="ExternalOutput")

    EXP = mybir.ActivationFunctionType.Exp

    with SafeTileContext(nc) as tc:
        with (
            nc.allow_low_precision("bf16 operands / fp32 psum accumulate"),
            tc.tile_pool(name="const", bufs=1) as const,
            tc.tile_pool(name="seq", bufs=1) as seq,
            tc.tile_pool(name="work", bufs=1) as work,
            tc.tile_pool(name="psA", bufs=2, space="PSUM") as psA,
            tc.tile_pool(name="psS", bufs=3, space="PSUM") as psS,
            tc.tile_pool(name="psY", bufs=3, space="PSUM") as psY,
        ):
            # ---------------- persistent constants
            # DMA order matters for the startup bubble: block 0's critical
            # path is wkv+xT(block0, emitted in the block loop) -> wq -> RoPE
            # tables; wo is only needed one block later.
            wq_sb = const.tile([128, 16 * 512], BF, tag="wq")
            wkv_sb = const.tile([128, 16 * 256], BF, tag="wkv")
            wo_sb = const.tile([128, 4 * 2048], BF, tag="wo")
            bq_sb = const.tile([128, 4], F32, tag="bq")
            bkv_sb = const.tile([128, 2], F32, tag="bkv")
            bvb_sb = const.tile([128, 128], F32, tag="bvb")
            cos_sb = const.tile([128, s], BF, tag="cos")
            sin_sb = const.tile([128, s], BF, tag="sin")
            nc.sync.dma_start(bq_sb[:], bq_d[:])
            nc.sync.dma_start(bkv_sb[:], bkv_d[:])
            nc.sync.dma_start(bvb_sb[:], bvb_d[:])
            for kt in range(16):
                nc.sync.dma_start(wkv_sb[:, kt * 256:(kt + 1) * 256],
                                  wkv_d[kt * 128:(kt + 1) * 128, :])
                nc.sync.dma_start(wq_sb[:, kt * 512:(kt + 1) * 512],
                                  wq_d[kt * 128:(kt + 1) * 128, :])
            nc.sync.dma_start(cos_sb[:], cos_d[:])
            nc.sync.dma_start(sin_sb[:], sin_d[:])
            for ft in range(4):
                nc.sync.dma_start(wo_sb[:, ft * 2048:(ft + 1) * 2048],
                                  wo_d[ft * 128:(ft + 1) * 128, :])

            stage = const.tile([128, 128], F32, tag="stage")
            make_identity(nc, stage[:])
            ident = const.tile([128, 128], BF, tag="ident")
            nc.vector.tensor_copy(ident[:], stage[:])
            # m_up[p, j] = 1 if j >= p (upper incl diag): current-block diagonal
            nc.gpsimd.memset(stage[:], 1.0)
            nc.gpsimd.affine_select(
                out=stage[:], in_=stage[:], fill=0.0,
                compare_op=mybir.AluOpType.is_ge,
                base=0, channel_multiplier=-1, pattern=[[1, 128]])
            m_up = const.tile([128, 128], BF, tag="m_up")
            nc.vector.tensor_copy(m_up[:], stage[:])
            # m_lo[p, j] = 1 if p >= j+1 (strict lower): prev-block halo
            nc.gpsimd.memset(stage[:], 1.0)
            nc.gpsimd.affine_select(
                out=stage[:], in_=stage[:], fill=0.0,
                compare_op=mybir.AluOpType.is_ge,
                base=-1, channel_multiplier=1, pattern=[[-1, 128]])
            m_lo = const.tile([128, 128], BF, tag="m_lo")
            nc.vector.tensor_copy(m_lo[:], stage[:])
            # m_row16[p] = 0 for p < 16 (meta tokens double-count guard)
            m_row16 = const.tile([128, 1], F32, tag="m_row16")
            nc.gpsimd.memset(m_row16[:], 1.0)
            nc.gpsimd.affine_select(
                out=m_row16[:], in_=m_row16[:], fill=0.0,
                compare_op=mybir.AluOpType.is_ge,
                base=-16, channel_multiplier=1, pattern=[[0, 1]])

            # ---------------- rolling K^T / V(+ones) state (3 blocks deep)
            kT_roll = seq.tile([128, 3 * 512], BF, tag="kTr")
            v_roll = seq.tile([128, 3 * 4 * VST], BF, tag="vr")
            kT_meta = seq.tile([128, 16], BF, tag="kTm")
            v_meta = seq.tile([16, VST], BF, tag="vm")
            for sl in range(12):
                nc.gpsimd.memset(v_roll[:, sl * VST + 128:sl * VST + 129], 1.0)
            nc.gpsimd.memset(v_meta[:, 128:129], 1.0)

            def rope_into(dst, src_ps, bias_col):
                """dst (sbuf bf16 [128,512]) = rope(src_ps + bias).
                src_ps is fp32 psum in [d, tok] quadrant-paired layout.
                Pool cannot touch PSUM, so psum-reading ops sit on DVE."""
                nc.vector.tensor_scalar_add(src_ps, src_ps, bias_col)
                qs = work.tile([128, 512], F32, tag="qs", bufs=2)
                nc.vector.stream_shuffle(qs[:], src_ps, SHUF_ROT16)
                tmp = work.tile([128, 512], BF, tag="tmp", bufs=2)
                nc.gpsimd.tensor_mul(tmp[:], qs[:], sin_t)
                nc.vector.tensor_mul(dst, src_ps, cos_t)
                nc.gpsimd.tensor_add(dst, dst, tmp[:])

            def fetch_xT(iblk):
                xT = work.tile([128, 16 * 512], BF, tag="xT", bufs=3)
                for et in range(16):
                    nc.sync.dma_start(xT[:, et * 512:(et + 1) * 512],
                                      xT_d[et * 128:(et + 1) * 128,
                                           iblk * 512:(iblk + 1) * 512])
                return xT

            xt_next = None
            for _pass in range(passes):
                for i in range(nblk):
                    cur = (i % 3) * 512
                    prv = ((i - 1) % 3) * 512
                    curv = (i % 3) * 4 * VST
                    prvv = ((i - 1) % 3) * 4 * VST
                    cos_t = cos_sb[:, i * 512:(i + 1) * 512]
                    sin_t = sin_sb[:, i * 512:(i + 1) * 512]

                    # ======== phase A (xT arrives via the previous block's
                    # prefetch; enqueue the next block's before this block's
                    # y DMAs so a pass boundary doesn't stall on the queue)
                    xT = fetch_xT(i) if xt_next is None else xt_next
                    last = (_pass == passes - 1) and (i == nblk - 1)
                    xt_next = None if last else fetch_xT((i + 1) % nblk)

                    # K^T directly: lhsT = wk chunk (stationary), rhs = xT
                    k_ps = psA.tile([128, 512], F32, tag="qkv")
                    for et in range(16):
                        nc.tensor.matmul(k_ps[:], wkv_sb[:, et * 256:et * 256 + 128],
                                         xT[:, et * 512:(et + 1) * 512],
                                         start=(et == 0), stop=(et == 15))
                    rope_into(kT_roll[:, cur:cur + 512], k_ps[:], bkv_sb[:, 0:1])

                    # Q^T per head
                    qT_blk = work.tile([128, 4 * 512], BF, tag="qT", bufs=2)
                    for h in range(4):
                        q_ps = psA.tile([128, 512], F32, tag="qkv")
                        for et in range(16):
                            nc.tensor.matmul(
                                q_ps[:],
                                wq_sb[:, et * 512 + h * 128:et * 512 + (h + 1) * 128],
                                xT[:, et * 512:(et + 1) * 512],
                                start=(et == 0), stop=(et == 15))
                        rope_into(qT_blk[:, h * 512:(h + 1) * 512], q_ps[:],
                                  bq_sb[:, h:h + 1])

                    # V natural [tok, d]: lhsT = xT subtile (stationary), rhs = wv
                    v_ps = psA.tile([128, 512], F32, tag="qkv")
                    for st in range(4):
                        for et in range(16):
                            nc.tensor.matmul(
                                v_ps[:, st * 128:(st + 1) * 128],
                                xT[:, et * 512 + st * 128:et * 512 + (st + 1) * 128],
                                wkv_sb[:, et * 256 + 128:(et + 1) * 256],
                                start=(et == 0), stop=(et == 15))
                    for st in range(4):
                        nc.vector.tensor_add(
                            v_roll[:, curv + st * VST:curv + st * VST + 128],
                            v_ps[:, st * 128:(st + 1) * 128], bvb_sb[:])

                    if i == 0:
                        nc.gpsimd.tensor_copy(kT_meta[:], kT_roll[:, cur:cur + 16])
                        nc.gpsimd.tensor_copy(v_meta[0:16, 0:128],
                                              v_roll[0:16, curv:curv + 128])

                    # chunk slots in pT: 0..3 current block, 4..7 prev, 8 meta.
                    def half_chunks(i_, half):
                        if i_ == 0:
                            return ([0, 1], [], False) if half == 0 else ([0, 1, 2, 3], [], False)
                        if half == 0:
                            return ([0, 1], [4, 5, 6, 7], True)
                        return ([0, 1, 2, 3], [6, 7], True)

                    # ======== phase B: scores (per half) then AV (per subtile),
                    # software-pipelined one head ahead.
                    def emit_scores(h):
                        pT = work.tile([128, 9 * 512], BF, tag="pT", bufs=2)
                        for half in range(2):
                            t0 = half * 2
                            cc, bb, mm_ = half_chunks(i, half)
                            qTr = qT_blk[:, h * 512 + t0 * 128:h * 512 + (t0 + 2) * 128]
                            slots = cc + bb
                            pairs = [slots[j:j + 2] for j in range(0, len(slots), 2)]
                            for pr in pairs:
                                st_ps = psS.tile([128, 512], F32, tag="s")
                                for j, kc in enumerate(pr):
                                    if kc < 4:
                                        kch = kT_roll[:, cur + kc * 128:cur + (kc + 1) * 128]
                                    else:
                                        kch = kT_roll[:, prv + (kc - 4) * 128:prv + (kc - 3) * 128]
                                    nc.tensor.matmul(st_ps[:, j * 256:(j + 1) * 256],
                                                     kch, qTr, start=True, stop=True)
                                # exp into the pair's two slots (strided out AP)
                                dst = pT.rearrange("p (sl q) -> p sl q", sl=9)
                                nc.scalar.activation(
                                    dst[:, pr[0]:pr[0] + len(pr),
                                        half * 256:(half + 1) * 256],
                                    st_ps[:, 0:256 * len(pr)], EXP)
                        if i > 0:
                            sm_ps = psS.tile([16, 512], F32, tag="s")
                            nc.tensor.matmul(sm_ps[:], kT_meta[:],
                                             qT_blk[:, h * 512:(h + 1) * 512],
                                             start=True, stop=True)
                            nc.scalar.activation(
                                pT[0:16, 8 * 512:9 * 512], sm_ps[:], EXP)
                        # masks: only the chunks AV actually consumes need them
                        for t in range(4):
                            co = t * 128
                            nc.gpsimd.tensor_mul(pT[:, t * 512 + co:t * 512 + co + 128],
                                                 pT[:, t * 512 + co:t * 512 + co + 128],
                                                 m_up[:])
                            if i > 0:
                                sl = pT[:, (4 + t) * 512 + co:(4 + t) * 512 + co + 128]
                                nc.gpsimd.tensor_mul(sl, sl, m_lo[:])
                        if i == 1:
                            sl = pT[:, 4 * 512:4 * 512 + 128]
                            nc.gpsimd.tensor_scalar_mul(sl, sl, m_row16[:])
                        return pT

                    def emit_av(h, pT):
                        for t in range(4):
                            co = t * 128
                            o_ps = psS.tile([128, 132], F32, tag="s")
                            chunks = [(pT[:, c * 512 + co:c * 512 + co + 128],
                                       v_roll[:, curv + c * VST:curv + c * VST + 129])
                                      for c in range(0, t + 1)]
                            if i > 0:
                                chunks += [(pT[:, (4 + c) * 512 + co:(4 + c) * 512 + co + 128],
                                            v_roll[:, prvv + c * VST:prvv + c * VST + 129])
                                           for c in range(t, 4)]
                                chunks.append((pT[0:16, 8 * 512 + co:8 * 512 + co + 128],
                                               v_meta[0:16, 0:129]))
                            for ci, (pch, vch) in enumerate(chunks):
                                nc.tensor.matmul(o_ps[:, 0:129], pch, vch,
                                                 start=(ci == 0), stop=(ci == len(chunks) - 1))
                            r = work.tile([128, 1], F32, tag="r", bufs=2)
                            nc.vector.reciprocal(r[:], o_ps[:, 128:129])
                            o_sb = work.tile([128, 128], BF, tag="osb", bufs=2)
                            nc.vector.tensor_scalar_mul(o_sb[:], o_ps[:, 0:128], r[:])
                            ot_ps = psY.tile([128, 256], BF, tag="ty")
                            nc.tensor.transpose(ot_ps[:, 0:128], o_sb[:], ident[:])
                            dst = oT_blk[:, h * 512 + t * 128:h * 512 + (t + 1) * 128]
                            if t % 2 == 0:
                                nc.scalar.copy(dst, ot_ps[:, 0:128])
                            else:
                                nc.vector.tensor_copy(dst, ot_ps[:, 0:128])

                    oT_blk = work.tile([128, 4 * 512], BF, tag="oT", bufs=2)
                    pT_q = []
                    for h in range(4):
                        pT_q.append(emit_scores(h))
                        if h >= 1:
                            emit_av(h - 1, pT_q[h - 1])
                    emit_av(3, pT_q[3])

                    # ======== phase C (for the PREVIOUS block, pipelined)
                    def emit_phase_c(oT, iblk):
                        for t in range(4):
                            gt = iblk * 4 + t
                            y_sb = work.tile([128, 2048], F32, tag="ysb", bufs=2)
                            for es in range(4):
                                y_ps = psY.tile([128, 512], F32, tag="ty")
                                for h in range(4):
                                    nc.tensor.matmul(
                                        y_ps[:],
                                        oT[:, h * 512 + t * 128:h * 512 + (t + 1) * 128],
                                        wo_sb[:, h * 2048 + es * 512:h * 2048 + (es + 1) * 512],
                                        start=(h == 0), stop=(h == 3))
                                if es % 2 == 0:
                                    nc.vector.tensor_copy(
                                        y_sb[:, es * 512:(es + 1) * 512], y_ps[:])
                                else:
                                    nc.scalar.copy(
                                        y_sb[:, es * 512:(es + 1) * 512], y_ps[:])
                            nc.sync.dma_start(y_d[gt * 128:(gt + 1) * 128, :], y_sb[:])

                    if i > 0:
                        emit_phase_c(prev_oT, i - 1)
                    prev_oT = oT_blk
                emit_phase_c(prev_oT, nblk - 1)
    return nc


# ---------------------------------------------------------------- host side
def _rope_perm():
    """Column permutation for Q/K: slot d <- original head-dim column.
    Quadrant q holds freqs q*16..q*16+15; first 16 slots = x1 (even orig
    cols), next 16 = x2 (odd orig cols)."""
    perm = np.zeros(HD, dtype=np.int64)
    for d in range(HD):
        q, r = divmod(d, 32)
        f = q * 16 + (r % 16)
        perm[d] = 2 * f + (0 if r < 16 else 1)
    return perm


def _prep_core_inputs(x, Wq, bq, Wkv, bkv, nblk=NBLK):
    """Build the 8 per-core input maps (host-side sharding + RoPE tables)."""
    import ml_dtypes

    s = nblk * BLK
    scale = 1.0 / np.sqrt(HD)
    perm = _rope_perm()

    # RoPE tables in [d, tok] layout matching the quadrant pairing;
    # sin sign-folded: rows r%32<16 get -sin.
    pos = np.arange(s, dtype=np.float64)
    inv_freq = 1.0 / (ROPE_BASE ** (np.arange(0, HD, 2, dtype=np.float64) / HD))
    f_of_d = np.array([(d // 32) * 16 + (d % 32) % 16 for d in range(HD)])
    ang = inv_freq[f_of_d][:, None] * pos[None, :]          # (d, s)
    cos_t = np.cos(ang)
    sgn = np.where((np.arange(HD) % 32) < 16, -1.0, 1.0)
    sin_t = sgn[:, None] * np.sin(ang)
    cos_c = np.ascontiguousarray(cos_t).astype(ml_dtypes.bfloat16)
    sin_c = np.ascontiguousarray(sin_t).astype(ml_dtypes.bfloat16)

    in_maps = []
    for b in range(2):
        xT_b = np.ascontiguousarray(
            np.asarray(x[b, :s]).T).astype(ml_dtypes.bfloat16)
        for g in range(N_KV):
            q_cols = []
            bq_cols = []
            for hl in range(NQH):
                hg = g * NQH + hl
                q_cols.append(Wq[:, hg * HD + perm] * scale)
                bq_cols.append(bq[hg * HD + perm] * scale)
            wq_s = np.concatenate(q_cols, axis=1)
            bq_c = np.stack(bq_cols, axis=1).astype(np.float32)   # (128, 4)
            wk = Wkv[:, g * HD + perm]
            wv = Wkv[:, N_KV * HD + g * HD:N_KV * HD + (g + 1) * HD]
            wkv_s = np.concatenate([wk, wv], axis=1)
            bk = bkv[g * HD + perm]
            bv = bkv[N_KV * HD + g * HD:N_KV * HD + (g + 1) * HD]
            bkv_c = np.stack([bk, bv], axis=1).astype(np.float32)  # (128, 2)
            in_maps.append({
                "xt": xT_b,
                "wq": np.ascontiguousarray(wq_s).astype(ml_dtypes.bfloat16),
                "wkv": np.ascontiguousarray(wkv_s).astype(ml_dtypes.bfloat16),
                "bqc": np.ascontiguousarray(bq_c),
                "bkvc": np.ascontiguousarray(bkv_c),
                "bvb": np.tile(bv.astype(np.float32)[None, :], (128, 1)),
                "cosd": cos_c,
                "sind": sin_c,
            })
    return in_maps


_RUNNER_CACHE = {}


class _Runner:
    """Builds the 8-core shard_map'd executable for build_nc() once, and runs
    it on device arrays (mirrors concourse.bass2jax.run_bass_via_pjrt, minus
    donation so timing loops can reuse resident inputs)."""

    N_CORES = 8

    def __init__(self, nblk=NBLK, passes=1):
        import jax
        from jax.sharding import Mesh, PartitionSpec, NamedSharding
        from jax.experimental.shard_map import shard_map
        import concourse.mybir as _mybir
        from concourse import bass2jax

        bass2jax.install_neuronx_cc_hook()
        self.jax = jax
        self.nc = build_nc(nblk, passes=passes)
        nc = self.nc
        in_names, out_names, out_avals, zero_outs = [], [], [], []
        partition_name = nc.partition_id_tensor.name if nc.partition_id_tensor else None
        for alloc in nc.m.functions[0].allocations:
            if not isinstance(alloc, _mybir.MemoryLocationSet):
                continue
            name = alloc.memorylocations[0].name
            if alloc.kind == "ExternalInput":
                if name != partition_name:
                    in_names.append(name)
            elif alloc.kind == "ExternalOutput":
                out_avals.append(jax.core.ShapedArray(
                    tuple(alloc.tensor_shape), _mybir.dt.np(alloc.dtype)))
                zero_outs.append(np.zeros(tuple(alloc.tensor_shape),
                                          _mybir.dt.np(alloc.dtype)))
                out_names.append(name)
        n_params = len(in_names)
        all_in_names = list(in_names) + list(out_names)
        if partition_name is not None:
            all_in_names.append(partition_name)
        self.in_names, self.out_names = in_names, out_names
        self.out_shapes = [a.shape for a in out_avals]

        def _body(*args):
            operands = list(args)
            if partition_name is not None:
                operands.append(bass2jax.partition_id_tensor())
            outs = bass2jax._bass_exec_p.bind(
                *operands,
                out_avals=tuple(out_avals),
                in_names=tuple(all_in_names),
                out_names=tuple(out_names),
                lowering_input_output_aliases=(),
                sim_require_finite=True,
                sim_require_nnan=True,
                nc=nc,
            )
            return tuple(outs)

        devices = jax.devices()[:self.N_CORES]
        self.mesh = Mesh(np.asarray(devices), ("core",))
        self.spec = PartitionSpec("core")
        self.sharding = NamedSharding(self.mesh, self.spec)
        nin = n_params + len(out_names)
        self.fn = jax.jit(
            shard_map(_body, mesh=self.mesh, in_specs=(self.spec,) * nin,
                      out_specs=(self.spec,) * len(out_names), check_rep=False),
            keep_unused=True,
        )
        self.zero_outs = zero_outs
        self._n_params = n_params

    def time_exec_vs(self, other, dev_args, iters=30):
        """Per-pass exec time: wall delta between this (1-pass) and `other`
        (N-pass) executables over identical device args.  Uses the median of
        PAIRED per-iteration differences — the dispatch wall noise (~ms) is
        mostly common-mode within an iteration."""
        import statistics
        import time as _time
        self.run(dev_args); other.run(dev_args)
        deltas = []
        for _ in range(iters):
            t0 = _time.perf_counter(); self.run(dev_args)
            a = _time.perf_counter() - t0
            t0 = _time.perf_counter(); other.run(dev_args)
            deltas.append(_time.perf_counter() - t0 - a)
        return statistics.median(deltas)

    def device_args(self, in_maps):
        cat = [np.concatenate([np.asarray(in_maps[c][n]) for c in range(self.N_CORES)],
                              axis=0) for n in self.in_names]
        cat += [np.zeros((self.N_CORES * z.shape[0], *z.shape[1:]), z.dtype)
                for z in self.zero_outs]
        return [self.jax.device_put(a, self.sharding) for a in cat]

    def run(self, dev_args):
        outs = self.fn(*dev_args)
        self.jax.block_until_ready(outs)
        return outs

    def results(self, outs):
        per_core = []
        for c in range(self.N_CORES):
            m = {}
            for i, name in enumerate(self.out_names):
                sh = self.out_shapes[i]
                m[name] = np.asarray(outs[i]).reshape(self.N_CORES, *sh)[c]
            per_core.append(m)
        return per_core


def _get_runner(nblk=NBLK):
    if nblk not in _RUNNER_CACHE:
        _RUNNER_CACHE[nblk] = _Runner(nblk)
    return _RUNNER_CACHE[nblk]


def _make_in_maps(x, Wq, bq, Wkv, bkv, Wo, nblk=NBLK):
    import ml_dtypes
    in_maps = _prep_core_inputs(x, Wq, bq, Wkv, bkv, nblk=nblk)
    for c in range(8):
        g = c % N_KV
        in_maps[c]["wo"] = np.ascontiguousarray(
            Wo[g * NQH * HD:(g + 1) * NQH * HD, :]).astype(ml_dtypes.bfloat16)
    return in_maps


def run_cores(x, Wq, bq, Wkv, bkv, Wo, bo, nblk=NBLK, time_iters=0):
    """Run the 8-core SPMD kernel; returns (y_full, exec_seconds_best)."""
    import time as _time

    s = nblk * BLK
    runner = _get_runner(nblk)
    in_maps = _make_in_maps(x, Wq, bq, Wkv, bkv, Wo, nblk=nblk)
    dev_args = runner.device_args(in_maps)
    outs = runner.run(dev_args)          # compile + first run
    best = None
    if time_iters:
        times = []
        for _ in range(time_iters):
            t0 = _time.perf_counter()
            runner.run(dev_args)
            times.append(_time.perf_counter() - t0)
        best = min(times)
    res = runner.results(outs)

    B = x.shape[0]
    y = np.zeros((B, s, E), dtype=np.float32)
    for b in range(B):
        acc = res[b * N_KV]["y"].astype(np.float32).copy()
        for g in range(1, N_KV):
            acc += res[b * N_KV + g]["y"]
        y[b] = acc + bo[None, :].astype(np.float32)
    return y, best


def kernel(x, Wq, bq, Wkv, bkv, Wo, bo):
    x = np.asarray(x, dtype=np.float32)
    Wq = np.asarray(Wq, dtype=np.float32)
    bq = np.asarray(bq, dtype=np.float32)
    Wkv = np.asarray(Wkv, dtype=np.float32)
    bkv = np.asarray(bkv, dtype=np.float32)
    Wo = np.asarray(Wo, dtype=np.float32)
    bo = np.asarray(bo, dtype=np.float32)
    y, _ = run_cores(x, Wq, bq, Wkv, bkv, Wo, bo)
    return y


# revision 19
# speedup vs baseline: 1.0499x; 1.0499x over previous
"""Trainium2 Bass kernel for nn_CausalRoPEAttention (windowed causal attention
with meta tokens, RoPE, GQA) on 8 NeuronCores.

Sharding: core c = b*4 + g  (b in {0,1} batch, g in {0..3} kv-head group).
Each core computes, for its batch b and its 4 query heads (= one GQA group):
  Q/K/V projections over full E, RoPE, windowed attention, and a partial
  output projection against its 512-row slice of Wo.  The host sums the 4
  partials per batch and adds bo.

v3 layout:
 - x arrives host-pre-transposed AND pre-bf16 as xT (E, S); Q and K are
   computed directly in [d, tok] orientation (weights stationary), so no
   on-device transposes of x / q / k at all.
 - RoPE runs in [d, tok] orientation: Q/K weight columns are host-permuted so
   each rotate-half partner sits 16 partitions away inside the same
   32-partition quadrant, making the rotation one DVE stream_shuffle; the
   cos / (sign-folded) sin tables are host-precomputed in the same layout.
 - V keeps natural [tok, d] orientation (x-tile stationary, bf16), and every
   V chunk carries an appended ones-column, so the attention AV matmul
   (query-major: p^T stationary, [V|1] moving) accumulates the softmax
   denominator for free in output column 128.  Normalization is then a
   per-partition reciprocal + scalar multiply; the row-sum ones-matmuls and
   the 1/s outer-product broadcast of v2 are gone.
 - Elementwise work is spread across Pool (gpsimd) / DVE / Act explicitly.
   1/sqrt(hd) is folded into Wq host-side.
"""

import numpy as np

import concourse.bass as bass
import concourse.mybir as mybir
import concourse.tile as tile
from concourse.masks import make_identity
from concourse.vector_clock import ScopedClock

# ---------------------------------------------------------------- constants
S = 4096
E = 2048
HD = 128
N_HEADS = 16
N_KV = 4
NQH = 4          # q heads per core
BLK = 512        # window size == query block
NBLK = S // BLK
META = 16
ROPE_BASE = 10000.0
F32 = mybir.dt.float32
BF = mybir.dt.bfloat16

VST = 132        # v_roll per-chunk column stride (128 v + 1 ones + 3 pad)
SHUF_ROT16 = [(i + 16) % 32 for i in range(32)]   # swap 16-halves per quadrant


class SafeTileContext(tile.TileContext):
    """TileContext whose final drain splits sem waits across nop instructions:
    this walrus caps non-EventSemaphore instructions at 1 sync wait."""

    def _drain_and_barrier(self, tick_clock, wait_clock):
        nc = self.nc
        dummy = nc.sync.nop(nofuse=True)
        wait_clock.add_sem_waits(dummy.ins, ScopedClock({None: tick_clock.global_clock}))
        waits = list(dummy.ins.sync_info.on_wait)
        if len(waits) > 1:
            dummy.ins.sync_info = mybir.SyncInfo(on_wait=waits[:1], on_update=[])
            for w in waits[1:]:
                nop = nc.sync.nop(nofuse=True)
                nop.ins.sync_info = mybir.SyncInfo(on_wait=[w], on_update=[])
        nc.sync.drain()
        nc.all_engine_barrier()
        popped = nc._tile_sem_poison_stack.pop()
        assert popped is self._sem_poison
        nc.clear_and_free_semaphores(list(self.sems.allocated().values()))
        nc.all_engine_barrier()
        self._split_excess_waits()

    def _split_excess_waits(self):
        """walrus (this version) allows 1 sync wait per instruction (2 on
        EventSemaphore); hoist extras onto preceding same-engine nops."""
        nc = self.nc
        for fn in nc.m.functions:
            for bb in fn.blocks:
                out = []
                changed = False
                for inst in bb.instructions:
                    si = inst.sync_info
                    waits = list(si.on_wait) if si is not None else []
                    cap = 2 if isinstance(inst, mybir.InstEventSemaphore) else 1
                    if len(waits) > cap:
                        changed = True
                        for w in waits[:-cap]:
                            nop = mybir.InstNoOp(
                                name=nc.get_next_instruction_name(), ins=[], outs=[])
                            nop.engine = inst.engine
                            nop.sync_info = mybir.SyncInfo(on_wait=[w], on_update=[])
                            nc.register_instruction(nop, overwrite=True)
                            out.append(nop)
                        inst.sync_info = mybir.SyncInfo(
                            on_wait=waits[-cap:], on_update=list(si.on_update))
                    out.append(inst)
                if changed:
                    bb.instructions = out


# ---------------------------------------------------------------- device IR
def build_nc(nblk=NBLK, passes=1):
    nc = bass.Bass()
    s = nblk * BLK

    xT_d = nc.dram_tensor("xt", (E, s), BF, kind="ExternalInput")
    wq_d = nc.dram_tensor("wq", (E, NQH * HD), BF, kind="ExternalInput")
    wkv_d = nc.dram_tensor("wkv", (E, 2 * HD), BF, kind="ExternalInput")
    wo_d = nc.dram_tensor("wo", (NQH * HD, E), BF, kind="ExternalInput")
    bq_d = nc.dram_tensor("bqc", (128, NQH), F32, kind="ExternalInput")
    bkv_d = nc.dram_tensor("bkvc", (128, 2), F32, kind="ExternalInput")
    bvb_d = nc.dram_tensor("bvb", (128, 128), F32, kind="ExternalInput")
    cos_d = nc.dram_tensor("cosd", (128, s), BF, kind="ExternalInput")
    sin_d = nc.dram_tensor("sind", (128, s), BF, kind="ExternalInput")
    y_d = nc.dram_tensor("y", (s, E), F32, kind="ExternalOutput")

    EXP = mybir.ActivationFunctionType.Exp

    with SafeTileContext(nc) as tc:
        with (
            nc.allow_low_precision("bf16 operands / fp32 psum accumulate"),
            tc.tile_pool(name="const", bufs=1) as const,
            tc.tile_pool(name="seq", bufs=1) as seq,
            tc.tile_pool(name="work", bufs=1) as work,
            tc.tile_pool(name="psA", bufs=2, space="PSUM") as psA,
            tc.tile_pool(name="psS", bufs=3, space="PSUM") as psS,
            tc.tile_pool(name="psY", bufs=3, space="PSUM") as psY,
        ):
            # ---------------- persistent constants
            # DMA order matters for the startup bubble: block 0's critical
            # path is wkv+xT(block0, emitted in the block loop) -> wq -> RoPE
            # tables; wo is only needed one block later.
            wq_sb = const.tile([128, 16 * 512], BF, tag="wq")
            wkv_sb = const.tile([128, 16 * 256], BF, tag="wkv")
            wo_sb = const.tile([128, 4 * 2048], BF, tag="wo")
            bq_sb = const.tile([128, 4], F32, tag="bq")
            bkv_sb = const.tile([128, 2], F32, tag="bkv")
            bvb_sb = const.tile([128, 128], F32, tag="bvb")
            cos_sb = const.tile([128, s], BF, tag="cos")
            sin_sb = const.tile([128, s], BF, tag="sin")
            nc.sync.dma_start(bq_sb[:], bq_d[:])
            nc.sync.dma_start(bkv_sb[:], bkv_d[:])
            nc.sync.dma_start(bvb_sb[:], bvb_d[:])
            for kt in range(16):
                nc.sync.dma_start(wkv_sb[:, kt * 256:(kt + 1) * 256],
                                  wkv_d[kt * 128:(kt + 1) * 128, :])
                nc.sync.dma_start(wq_sb[:, kt * 512:(kt + 1) * 512],
                                  wq_d[kt * 128:(kt + 1) * 128, :])
            nc.sync.dma_start(cos_sb[:], cos_d[:])
            nc.sync.dma_start(sin_sb[:], sin_d[:])
            for ft in range(4):
                nc.sync.dma_start(wo_sb[:, ft * 2048:(ft + 1) * 2048],
                                  wo_d[ft * 128:(ft + 1) * 128, :])

            stage = const.tile([128, 128], F32, tag="stage")
            make_identity(nc, stage[:])
            ident = const.tile([128, 128], BF, tag="ident")
            nc.vector.tensor_copy(ident[:], stage[:])
            # m_up[p, j] = 1 if j >= p (upper incl diag): current-block diagonal
            nc.gpsimd.memset(stage[:], 1.0)
            nc.gpsimd.affine_select(
                out=stage[:], in_=stage[:], fill=0.0,
                compare_op=mybir.AluOpType.is_ge,
                base=0, channel_multiplier=-1, pattern=[[1, 128]])
            m_up = const.tile([128, 128], BF, tag="m_up")
            nc.vector.tensor_copy(m_up[:], stage[:])
            # m_lo[p, j] = 1 if p >= j+1 (strict lower): prev-block halo
            nc.gpsimd.memset(stage[:], 1.0)
            nc.gpsimd.affine_select(
                out=stage[:], in_=stage[:], fill=0.0,
                compare_op=mybir.AluOpType.is_ge,
                base=-1, channel_multiplier=1, pattern=[[-1, 128]])
            m_lo = const.tile([128, 128], BF, tag="m_lo")
            nc.vector.tensor_copy(m_lo[:], stage[:])
            # m_row16[p] = 0 for p < 16 (meta tokens double-count guard)
            m_row16 = const.tile([128, 1], F32, tag="m_row16")
            nc.gpsimd.memset(m_row16[:], 1.0)
            nc.gpsimd.affine_select(
                out=m_row16[:], in_=m_row16[:], fill=0.0,
                compare_op=mybir.AluOpType.is_ge,
                base=-16, channel_multiplier=1, pattern=[[0, 1]])

            # ---------------- rolling K^T / V(+ones) state (3 blocks deep)
            kT_roll = seq.tile([128, 3 * 512], BF, tag="kTr")
            v_roll = seq.tile([128, 3 * 4 * VST], BF, tag="vr")
            kT_meta = seq.tile([128, 16], BF, tag="kTm")
            v_meta = seq.tile([16, VST], BF, tag="vm")
            for sl in range(12):
                nc.gpsimd.memset(v_roll[:, sl * VST + 128:sl * VST + 129], 1.0)
            nc.gpsimd.memset(v_meta[:, 128:129], 1.0)

            def rope_into(dst, src_ps, bias_col):
                """dst (sbuf bf16 [128,512]) = rope(src_ps + bias).
                src_ps is fp32 psum in [d, tok] quadrant-paired layout.
                Pool cannot touch PSUM, so psum-reading ops sit on DVE."""
                nc.vector.tensor_scalar_add(src_ps, src_ps, bias_col)
                qs = work.tile([128, 512], F32, tag="qs", bufs=2)
                nc.vector.stream_shuffle(qs[:], src_ps, SHUF_ROT16)
                tmp = work.tile([128, 512], BF, tag="tmp", bufs=2)
                nc.gpsimd.tensor_mul(tmp[:], qs[:], sin_t)
                nc.vector.tensor_mul(dst, src_ps, cos_t)
                nc.gpsimd.tensor_add(dst, dst, tmp[:])

            def fetch_xT(iblk):
                xT = work.tile([128, 16 * 512], BF, tag="xT", bufs=3)
                for et in range(16):
                    nc.sync.dma_start(xT[:, et * 512:(et + 1) * 512],
                                      xT_d[et * 128:(et + 1) * 128,
                                           iblk * 512:(iblk + 1) * 512])
                return xT

            xt_next = None
            for _pass in range(passes):
                for i in range(nblk):
                    cur = (i % 3) * 512
                    prv = ((i - 1) % 3) * 512
                    curv = (i % 3) * 4 * VST
                    prvv = ((i - 1) % 3) * 4 * VST
                    cos_t = cos_sb[:, i * 512:(i + 1) * 512]
                    sin_t = sin_sb[:, i * 512:(i + 1) * 512]

                    # ======== phase A (xT arrives via the previous block's
                    # prefetch; enqueue the next block's before this block's
                    # y DMAs so a pass boundary doesn't stall on the queue)
                    xT = fetch_xT(i) if xt_next is None else xt_next
                    last = (_pass == passes - 1) and (i == nblk - 1)
                    xt_next = None if last else fetch_xT((i + 1) % nblk)

                    # K^T directly: lhsT = wk chunk (stationary), rhs = xT
                    k_ps = psA.tile([128, 512], F32, tag="qkv")
                    for et in range(16):
                        nc.tensor.matmul(k_ps[:], wkv_sb[:, et * 256:et * 256 + 128],
                                         xT[:, et * 512:(et + 1) * 512],
                                         start=(et == 0), stop=(et == 15))
                    rope_into(kT_roll[:, cur:cur + 512], k_ps[:], bkv_sb[:, 0:1])

                    # Q^T per head
                    qT_blk = work.tile([128, 4 * 512], BF, tag="qT", bufs=2)
                    for h in range(4):
                        q_ps = psA.tile([128, 512], F32, tag="qkv")
                        for et in range(16):
                            nc.tensor.matmul(
                                q_ps[:],
                                wq_sb[:, et * 512 + h * 128:et * 512 + (h + 1) * 128],
                                xT[:, et * 512:(et + 1) * 512],
                                start=(et == 0), stop=(et == 15))
                        rope_into(qT_blk[:, h * 512:(h + 1) * 512], q_ps[:],
                                  bq_sb[:, h:h + 1])

                    # V natural [tok, d]: lhsT = xT subtile (stationary), rhs = wv
                    v_ps = psA.tile([128, 512], F32, tag="qkv")
                    for st in range(4):
                        for et in range(16):
                            nc.tensor.matmul(
                                v_ps[:, st * 128:(st + 1) * 128],
                                xT[:, et * 512 + st * 128:et * 512 + (st + 1) * 128],
                                wkv_sb[:, et * 256 + 128:(et + 1) * 256],
                                start=(et == 0), stop=(et == 15))
                    for st in range(4):
                        nc.vector.tensor_add(
                            v_roll[:, curv + st * VST:curv + st * VST + 128],
                            v_ps[:, st * 128:(st + 1) * 128], bvb_sb[:])

                    if i == 0:
                        nc.gpsimd.tensor_copy(kT_meta[:], kT_roll[:, cur:cur + 16])
                        nc.gpsimd.tensor_copy(v_meta[0:16, 0:128],
                                              v_roll[0:16, curv:curv + 128])

                    # chunk slots in pT: 0..3 current block, 4..7 prev, 8 meta.
                    def half_chunks(i_, half):
                        if i_ == 0:
                            return ([0, 1], [], False) if half == 0 else ([0, 1, 2, 3], [], False)
                        if half == 0:
                            return ([0, 1], [4, 5, 6, 7], True)
                        return ([0, 1, 2, 3], [6, 7], True)

                    # ======== phase B: scores (per half) then AV (per subtile),
                    # software-pipelined one head ahead.
                    def kslot(kc):
                        if kc < 4:
                            return kT_roll[:, cur + kc * 128:cur + (kc + 1) * 128]
                        return kT_roll[:, prv + (kc - 4) * 128:prv + (kc - 3) * 128]

                    def emit_scores(h):
                        pT = work.tile([128, 9 * 512], BF, tag="pT", bufs=2)
                        dst = pT.rearrange("p (sl q) -> p sl q", sl=9)
                        for half in range(2):
                            t0 = half * 2
                            qTr = qT_blk[:, h * 512 + t0 * 128:h * 512 + (t0 + 2) * 128]
                            if i == 0:
                                cc, bb, mm_ = half_chunks(i, half)
                                slots = cc + bb
                                pairs = [slots[j:j + 2] for j in range(0, len(slots), 2)]
                                ragged = None
                            elif half == 0:
                                # slot 1 only feeds t=1 (hi cols), slot 4 only t=0 (lo)
                                pairs = [[0, 5], [6, 7]]
                                ragged = (1, 4)
                            else:
                                # slot 3 only feeds t=3 (hi cols), slot 6 only t=2 (lo)
                                pairs = [[0, 1], [2, 7]]
                                ragged = (3, 6)
                            for pr in pairs:
                                st_ps = psS.tile([128, 512], F32, tag="s")
                                for j, kc in enumerate(pr):
                                    nc.tensor.matmul(st_ps[:, j * 256:(j + 1) * 256],
                                                     kslot(kc), qTr, start=True, stop=True)
                                # exp into the pair's two slots (strided out AP)
                                nc.scalar.activation(
                                    dst[:, pr[0]:pr[1] + 1:pr[1] - pr[0],
                                        half * 256:(half + 1) * 256],
                                    st_ps[:], EXP)
                            if ragged is not None:
                                hi, lo = ragged
                                st_ps = psS.tile([128, 256], F32, tag="s")
                                nc.tensor.matmul(st_ps[:, 0:128], kslot(hi),
                                                 qTr[:, 128:256], start=True, stop=True)
                                nc.tensor.matmul(st_ps[:, 128:256], kslot(lo),
                                                 qTr[:, 0:128], start=True, stop=True)
                                base = half * 256
                                nc.scalar.activation(
                                    pT[:, hi * 512 + base + 128:hi * 512 + base + 256],
                                    st_ps[:, 0:128], EXP)
                                nc.scalar.activation(
                                    pT[:, lo * 512 + base:lo * 512 + base + 128],
                                    st_ps[:, 128:256], EXP)
                        if i > 0:
                            sm_ps = psS.tile([16, 512], F32, tag="s")
                            nc.tensor.matmul(sm_ps[:], kT_meta[:],
                                             qT_blk[:, h * 512:(h + 1) * 512],
                                             start=True, stop=True)
                            nc.scalar.activation(
                                pT[0:16, 8 * 512:9 * 512], sm_ps[:], EXP)
                        # masks: only the chunks AV actually consumes need them
                        for t in range(4):
                            co = t * 128
                            nc.gpsimd.tensor_mul(pT[:, t * 512 + co:t * 512 + co + 128],
                                                 pT[:, t * 512 + co:t * 512 + co + 128],
                                                 m_up[:])
                            if i > 0:
                                sl = pT[:, (4 + t) * 512 + co:(4 + t) * 512 + co + 128]
                                nc.gpsimd.tensor_mul(sl, sl, m_lo[:])
                        if i == 1:
                            sl = pT[:, 4 * 512:4 * 512 + 128]
                            nc.gpsimd.tensor_scalar_mul(sl, sl, m_row16[:])
                        return pT

                    def emit_av(h, pT):
                        for t in range(4):
                            co = t * 128
                            o_ps = psS.tile([128, 132], F32, tag="s")
                            chunks = [(pT[:, c * 512 + co:c * 512 + co + 128],
                                       v_roll[:, curv + c * VST:curv + c * VST + 129])
                                      for c in range(0, t + 1)]
                            if i > 0:
                                chunks += [(pT[:, (4 + c) * 512 + co:(4 + c) * 512 + co + 128],
                                            v_roll[:, prvv + c * VST:prvv + c * VST + 129])
                                           for c in range(t, 4)]
                                chunks.append((pT[0:16, 8 * 512 + co:8 * 512 + co + 128],
                                               v_meta[0:16, 0:129]))
                            for ci, (pch, vch) in enumerate(chunks):
                                nc.tensor.matmul(o_ps[:, 0:129], pch, vch,
                                                 start=(ci == 0), stop=(ci == len(chunks) - 1))
                            r = work.tile([128, 1], F32, tag="r", bufs=2)
                            nc.vector.reciprocal(r[:], o_ps[:, 128:129])
                            o_sb = work.tile([128, 128], BF, tag="osb", bufs=2)
                            nc.vector.tensor_scalar_mul(o_sb[:], o_ps[:, 0:128], r[:])
                            ot_ps = psY.tile([128, 256], BF, tag="ty")
                            nc.tensor.transpose(ot_ps[:, 0:128], o_sb[:], ident[:])
                            dst = oT_blk[:, h * 512 + t * 128:h * 512 + (t + 1) * 128]
                            if t % 2 == 0:
                                nc.scalar.copy(dst, ot_ps[:, 0:128])
                            else:
                                nc.vector.tensor_copy(dst, ot_ps[:, 0:128])

                    oT_blk = work.tile([128, 4 * 512], BF, tag="oT", bufs=2)
                    pT_q = []
                    for h in range(4):
                        pT_q.append(emit_scores(h))
                        if h >= 1:
                            emit_av(h - 1, pT_q[h - 1])
                    emit_av(3, pT_q[3])

                    # ======== phase C (for the PREVIOUS block, pipelined)
                    def emit_phase_c(oT, iblk):
                        for t in range(4):
                            gt = iblk * 4 + t
                            y_sb = work.tile([128, 2048], F32, tag="ysb", bufs=2)
                            for es in range(4):
                                y_ps = psY.tile([128, 512], F32, tag="ty")
                                for h in range(4):
                                    nc.tensor.matmul(
                                        y_ps[:],
                                        oT[:, h * 512 + t * 128:h * 512 + (t + 1) * 128],
                                        wo_sb[:, h * 2048 + es * 512:h * 2048 + (es + 1) * 512],
                                        start=(h == 0), stop=(h == 3))
                                if es % 2 == 0:
                                    nc.vector.tensor_copy(
                                        y_sb[:, es * 512:(es + 1) * 512], y_ps[:])
                                else:
                                    nc.scalar.copy(
                                        y_sb[:, es * 512:(es + 1) * 512], y_ps[:])
                            nc.sync.dma_start(y_d[gt * 128:(gt + 1) * 128, :], y_sb[:])

                    if i > 0:
                        emit_phase_c(prev_oT, i - 1)
                    prev_oT = oT_blk
                emit_phase_c(prev_oT, nblk - 1)
    return nc


# ---------------------------------------------------------------- host side
def _rope_perm():
    """Column permutation for Q/K: slot d <- original head-dim column.
    Quadrant q holds freqs q*16..q*16+15; first 16 slots = x1 (even orig
    cols), next 16 = x2 (odd orig cols)."""
    perm = np.zeros(HD, dtype=np.int64)
    for d in range(HD):
        q, r = divmod(d, 32)
        f = q * 16 + (r % 16)
        perm[d] = 2 * f + (0 if r < 16 else 1)
    return perm


def _prep_core_inputs(x, Wq, bq, Wkv, bkv, nblk=NBLK):
    """Build the 8 per-core input maps (host-side sharding + RoPE tables)."""
    import ml_dtypes

    s = nblk * BLK
    scale = 1.0 / np.sqrt(HD)
    perm = _rope_perm()

    # RoPE tables in [d, tok] layout matching the quadrant pairing;
    # sin sign-folded: rows r%32<16 get -sin.
    pos = np.arange(s, dtype=np.float64)
    inv_freq = 1.0 / (ROPE_BASE ** (np.arange(0, HD, 2, dtype=np.float64) / HD))
    f_of_d = np.array([(d // 32) * 16 + (d % 32) % 16 for d in range(HD)])
    ang = inv_freq[f_of_d][:, None] * pos[None, :]          # (d, s)
    cos_t = np.cos(ang)
    sgn = np.where((np.arange(HD) % 32) < 16, -1.0, 1.0)
    sin_t = sgn[:, None] * np.sin(ang)
    cos_c = np.ascontiguousarray(cos_t).astype(ml_dtypes.bfloat16)
    sin_c = np.ascontiguousarray(sin_t).astype(ml_dtypes.bfloat16)

    in_maps = []
    for b in range(2):
        xT_b = np.ascontiguousarray(
            np.asarray(x[b, :s]).T).astype(ml_dtypes.bfloat16)
        for g in range(N_KV):
            q_cols = []
            bq_cols = []
            for hl in range(NQH):
                hg = g * NQH + hl
                q_cols.append(Wq[:, hg * HD + perm] * scale)
                bq_cols.append(bq[hg * HD + perm] * scale)
            wq_s = np.concatenate(q_cols, axis=1)
            bq_c = np.stack(bq_cols, axis=1).astype(np.float32)   # (128, 4)
            wk = Wkv[:, g * HD + perm]
            wv = Wkv[:, N_KV * HD + g * HD:N_KV * HD + (g + 1) * HD]
            wkv_s = np.concatenate([wk, wv], axis=1)
            bk = bkv[g * HD + perm]
            bv = bkv[N_KV * HD + g * HD:N_KV * HD + (g + 1) * HD]
            bkv_c = np.stack([bk, bv], axis=1).astype(np.float32)  # (128, 2)
            in_maps.append({
                "xt": xT_b,
                "wq": np.ascontiguousarray(wq_s).astype(ml_dtypes.bfloat16),
                "wkv": np.ascontiguousarray(wkv_s).astype(ml_dtypes.bfloat16),
                "bqc": np.ascontiguousarray(bq_c),
                "bkvc": np.ascontiguousarray(bkv_c),
                "bvb": np.tile(bv.astype(np.float32)[None, :], (128, 1)),
                "cosd": cos_c,
                "sind": sin_c,
            })
    return in_maps


_RUNNER_CACHE = {}


class _Runner:
    """Builds the 8-core shard_map'd executable for build_nc() once, and runs
    it on device arrays (mirrors concourse.bass2jax.run_bass_via_pjrt, minus
    donation so timing loops can reuse resident inputs)."""

    N_CORES = 8

    def __init__(self, nblk=NBLK, passes=1):
        import jax
        from jax.sharding import Mesh, PartitionSpec, NamedSharding
        from jax.experimental.shard_map import shard_map
        import concourse.mybir as _mybir
        from concourse import bass2jax

        bass2jax.install_neuronx_cc_hook()
        self.jax = jax
        self.nc = build_nc(nblk, passes=passes)
        nc = self.nc
        in_names, out_names, out_avals, zero_outs = [], [], [], []
        partition_name = nc.partition_id_tensor.name if nc.partition_id_tensor else None
        for alloc in nc.m.functions[0].allocations:
            if not isinstance(alloc, _mybir.MemoryLocationSet):
                continue
            name = alloc.memorylocations[0].name
            if alloc.kind == "ExternalInput":
                if name != partition_name:
                    in_names.append(name)
            elif alloc.kind == "ExternalOutput":
                out_avals.append(jax.core.ShapedArray(
                    tuple(alloc.tensor_shape), _mybir.dt.np(alloc.dtype)))
                zero_outs.append(np.zeros(tuple(alloc.tensor_shape),
                                          _mybir.dt.np(alloc.dtype)))
                out_names.append(name)
        n_params = len(in_names)
        all_in_names = list(in_names) + list(out_names)
        if partition_name is not None:
            all_in_names.append(partition_name)
        self.in_names, self.out_names = in_names, out_names
        self.out_shapes = [a.shape for a in out_avals]

        def _body(*args):
            operands = list(args)
            if partition_name is not None:
                operands.append(bass2jax.partition_id_tensor())
            outs = bass2jax._bass_exec_p.bind(
                *operands,
                out_avals=tuple(out_avals),
                in_names=tuple(all_in_names),
                out_names=tuple(out_names),
                lowering_input_output_aliases=(),
                sim_require_finite=True,
                sim_require_nnan=True,
                nc=nc,
            )
            return tuple(outs)

        devices = jax.devices()[:self.N_CORES]
        self.mesh = Mesh(np.asarray(devices), ("core",))
        self.spec = PartitionSpec("core")
        self.sharding = NamedSharding(self.mesh, self.spec)
        nin = n_params + len(out_names)
        self.fn = jax.jit(
            shard_map(_body, mesh=self.mesh, in_specs=(self.spec,) * nin,
                      out_specs=(self.spec,) * len(out_names), check_rep=False),
            keep_unused=True,
        )
        self.zero_outs = zero_outs
        self._n_params = n_params

    def time_exec_vs(self, other, dev_args, iters=30):
        """Per-pass exec time: wall delta between this (1-pass) and `other`
        (N-pass) executables over identical device args.  Uses the median of
        PAIRED per-iteration differences — the dispatch wall noise (~ms) is
        mostly common-mode within an iteration."""
        import statistics
        import time as _time
        self.run(dev_args); other.run(dev_args)
        deltas = []
        for _ in range(iters):
            t0 = _time.perf_counter(); self.run(dev_args)
            a = _time.perf_counter() - t0
            t0 = _time.perf_counter(); other.run(dev_args)
            deltas.append(_time.perf_counter() - t0 - a)
        return statistics.median(deltas)

    def device_args(self, in_maps):
        cat = [np.concatenate([np.asarray(in_maps[c][n]) for c in range(self.N_CORES)],
                              axis=0) for n in self.in_names]
        cat += [np.zeros((self.N_CORES * z.shape[0], *z.shape[1:]), z.dtype)
                for z in self.zero_outs]
        return [self.jax.device_put(a, self.sharding) for a in cat]

    def run(self, dev_args):
        outs = self.fn(*dev_args)
        self.jax.block_until_ready(outs)
        return outs

    def results(self, outs):
        per_core = []
        for c in range(self.N_CORES):
            m = {}
            for i, name in enumerate(self.out_names):
                sh = self.out_shapes[i]
                m[name] = np.asarray(outs[i]).reshape(self.N_CORES, *sh)[c]
            per_core.append(m)
        return per_core


def _get_runner(nblk=NBLK):
    if nblk not in _RUNNER_CACHE:
        _RUNNER_CACHE[nblk] = _Runner(nblk)
    return _RUNNER_CACHE[nblk]


def _make_in_maps(x, Wq, bq, Wkv, bkv, Wo, nblk=NBLK):
    import ml_dtypes
    in_maps = _prep_core_inputs(x, Wq, bq, Wkv, bkv, nblk=nblk)
    for c in range(8):
        g = c % N_KV
        in_maps[c]["wo"] = np.ascontiguousarray(
            Wo[g * NQH * HD:(g + 1) * NQH * HD, :]).astype(ml_dtypes.bfloat16)
    return in_maps


def run_cores(x, Wq, bq, Wkv, bkv, Wo, bo, nblk=NBLK, time_iters=0):
    """Run the 8-core SPMD kernel; returns (y_full, exec_seconds_best)."""
    import time as _time

    s = nblk * BLK
    runner = _get_runner(nblk)
    in_maps = _make_in_maps(x, Wq, bq, Wkv, bkv, Wo, nblk=nblk)
    dev_args = runner.device_args(in_maps)
    outs = runner.run(dev_args)          # compile + first run
    best = None
    if time_iters:
        times = []
        for _ in range(time_iters):
            t0 = _time.perf_counter()
            runner.run(dev_args)
            times.append(_time.perf_counter() - t0)
        best = min(times)
    res = runner.results(outs)

    B = x.shape[0]
    y = np.zeros((B, s, E), dtype=np.float32)
    for b in range(B):
        acc = res[b * N_KV]["y"].astype(np.float32).copy()
        for g in range(1, N_KV):
            acc += res[b * N_KV + g]["y"]
        y[b] = acc + bo[None, :].astype(np.float32)
    return y, best


def kernel(x, Wq, bq, Wkv, bkv, Wo, bo):
    x = np.asarray(x, dtype=np.float32)
    Wq = np.asarray(Wq, dtype=np.float32)
    bq = np.asarray(bq, dtype=np.float32)
    Wkv = np.asarray(Wkv, dtype=np.float32)
    bkv = np.asarray(bkv, dtype=np.float32)
    Wo = np.asarray(Wo, dtype=np.float32)
    bo = np.asarray(bo, dtype=np.float32)
    y, _ = run_cores(x, Wq, bq, Wkv, bkv, Wo, bo)
    return y
